# revision 1
# baseline (speedup 1.0000x reference)
"""Trainium2 Bass kernel for the gnn_message_passing NodeModel.

reference semantics:
    agg = segment_sum(edge_attr, edge_index[1], N)        # [N, 128]
    h   = silu(concat([x, agg, f]) @ W0 + b0)
    h   = silu(h @ W1 + b1)
    out = h @ W2 + b2

Strategy (edge-parallel, destination-bucketed, fully fused):
  * Host groups edges by destination block of W=64 nodes.  The 1568 node
    blocks (100000 padded to 1568*64) are dealt round-robin by edge count
    into 196 "slots" x 8 cores, so one SPMD program (identical instruction
    stream, per-core data) covers all cores with ~8% edge padding.
  * Device, per 128-edge tile: build a one-hot [edge, node_off] matrix with
    is_equal(iota, dest_off) and matmul-accumulate the W-node block
    aggregate transposed [d, node] in PSUM.  Edges sorted by destination
    make every tile target a single W-node block.  One-hot builds are
    batched TPB tiles per DVE tensor_tensor op in [p, (node, tile)] layout
    so the last AP dim is packed (step 1) -> DVE 2x perf mode.
  * Two slots share one [128,128] PSUM tile (two 64-col accumulation
    regions); one scalar-engine copy drains the pair to SBUF bf16.
  * The 3-layer MLP runs fused per 512-node group straight from SBUF
    (features-on-partitions), silu = z * sigmoid(z) on scalar+vector
    engines.  GPSIMD is avoided entirely (~1.8 us/op dispatch cost).
    Output written back transposed bf16; host un-transposes and upcasts.
  * All matmul operands bf16 (f32 PSUM accumulate): end-to-end rel err
    ~5e-3, and the kernel is HBM-bandwidth-bound, reading ~57 MB/core.
"""

import math
import os
import sys

import numpy as np

_TRN_REPO = "/opt/trn_rl_repo"
if _TRN_REPO not in sys.path:
    sys.path.insert(0, _TRN_REPO)

import ml_dtypes

P = 128
D = 128
F = 3
N = 100000
E = 1600000
NCORES = 8
NODES_PAD = 100352    # nodes incl. padding (multiple of 128*NCORES)

W = 64                # default node-block width
NBLK = NODES_PAD // W
SLOTS = NBLK // NCORES
NODES_C = SLOTS * W
CH = 64               # 128-edge tiles per attr DMA chunk (2 MB bf16)
GROUP_SLOTS = 512 // W
SPP = P // W
TPB = 1024 // W

BF16 = ml_dtypes.bfloat16


def set_w(w):
    """Reconfigure the node-block width (32/64/128)."""
    global W, NBLK, SLOTS, NODES_C, GROUP_SLOTS, SPP, TPB
    W = w
    NBLK = NODES_PAD // W
    SLOTS = NBLK // NCORES
    NODES_C = SLOTS * W
    GROUP_SLOTS = 512 // W
    SPP = P // W
    TPB = 1024 // W


# ---------------------------------------------------------------- host prep
def _prep(inputs):
    x = np.asarray(inputs["x"], np.float32)
    edge_index = np.asarray(inputs["edge_index"])
    edge_attr = np.asarray(inputs["edge_attr"], np.float32)
    f = np.asarray(inputs["f"], np.float32)
    W0 = np.asarray(inputs["W0"], np.float32)
    W1 = np.asarray(inputs["W1"], np.float32)
    W2 = np.asarray(inputs["W2"], np.float32)
    b0 = np.asarray(inputs["b0"], np.float32)
    b1 = np.asarray(inputs["b1"], np.float32)
    b2 = np.asarray(inputs["b2"], np.float32)

    dest = edge_index[1].astype(np.int64)
    shift = W.bit_length() - 1
    blk = dest >> shift
    off = (dest & (W - 1)).astype(np.int32)

    counts = np.bincount(blk, minlength=NBLK)
    rank = np.argsort(-counts, kind="stable")          # block ids, big first
    inv = np.empty(NBLK, np.int64)
    inv[rank] = np.arange(NBLK)
    slot_of_blk = inv // NCORES
    core_of_blk = inv % NCORES

    # tiles per slot: shared across cores (same NEFF), max over the slot's blocks
    TS = np.maximum(
        1, np.ceil(counts[rank].reshape(SLOTS, NCORES).max(axis=1) / P).astype(np.int64)
    )
    TT = int(TS.sum())
    NCH = math.ceil(TT / CH)
    ts_real = TS.copy()          # tiles that carry real edges (last slot excl. chunk pad)
    TS[-1] += NCH * CH - TT
    TT = NCH * CH
    cum = np.zeros(SLOTS, np.int64)
    cum[1:] = np.cumsum(TS)[:-1]

    order = np.argsort(blk, kind="stable")
    blk_sorted = blk[order]
    start = np.zeros(NBLK, np.int64)
    start[1:] = np.cumsum(counts)[:-1]
    within = np.arange(E, dtype=np.int64) - start[blk_sorted]
    rows = cum[slot_of_blk[blk_sorted]] * P + within
    cores = core_of_blk[blk_sorted]

    ea_bf = edge_attr.astype(BF16)
    xpad = np.zeros((NBLK * W, D), np.float32)
    xpad[:N] = x
    fpad = np.zeros((NBLK * W, F), np.float32)
    fpad[:N] = f
    xT_all = np.ascontiguousarray(xpad.T).astype(BF16).reshape(P, NBLK, W)
    fT_all = np.ascontiguousarray(fpad.T).astype(BF16).reshape(F, NBLK, W)

    w_shared = {
        "w0x": W0[:D].astype(BF16),
        "w0a": W0[D:2 * D].astype(BF16),
        "w0f": W0[2 * D:].astype(BF16),
        "w1": W1.astype(BF16),
        "w2": W2.astype(BF16),
        "b0": b0.reshape(P, 1).astype(np.float32),
        "b1": b1.reshape(P, 1).astype(np.float32),
        "b2": b2.reshape(P, 1).astype(np.float32),
    }

    in_maps = []
    blocks_per_core = []
    for c in range(NCORES):
        m = cores == c
        attr_pack = np.zeros((TT * P, D), BF16)
        attr_pack[rows[m]] = ea_bf[order[m]]
        dest_pack = np.full((TT * P,), -1.0, BF16)
        dest_pack[rows[m]] = off[order[m]].astype(BF16)
        attr_dma = np.ascontiguousarray(
            attr_pack.reshape(NCH, CH, P, D).transpose(0, 2, 1, 3)
        ).reshape(NCH, P, CH * D)
        dest_dma = np.ascontiguousarray(dest_pack.reshape(TT, P).T)
        blocks_c = rank[np.arange(SLOTS) * NCORES + c]
        blocks_per_core.append(blocks_c)
        xT_c = np.ascontiguousarray(xT_all[:, blocks_c, :]).reshape(P, NODES_C)
        fT_c = np.ascontiguousarray(fT_all[:, blocks_c, :]).reshape(F, NODES_C)
        im = {"attr": attr_dma, "dest": dest_dma, "xT": xT_c, "fT": fT_c}
        im.update(w_shared)
        in_maps.append(im)

    return in_maps, blocks_per_core, TS, cum, NCH, ts_real


# ---------------------------------------------------------------- device code
def _build(TS, cum, NCH, reps=1, ts_real=None, out_bf16=True, oh_layout="tj",
           bodies_per_iter=1, attr_bufs=4, oh_bufs=4):
    if ts_real is None:
        ts_real = TS
    import concourse.bass as bass
    import concourse.bacc as bacc
    import concourse.mybir as mybir
    import concourse.tile as tile

    bf = mybir.dt.bfloat16
    f32 = mybir.dt.float32
    out_dt = bf if out_bf16 else f32
    TT = int(TS.sum())

    nc = bacc.Bacc("TRN2", target_bir_lowering=False, debug=False, num_devices=NCORES)

    attr_d = nc.dram_tensor("attr", [NCH, P, CH * P], bf, kind="ExternalInput")
    dest_d = nc.dram_tensor("dest", [P, TT], bf, kind="ExternalInput")
    xT_d = nc.dram_tensor("xT", [P, NODES_C], bf, kind="ExternalInput")
    fT_d = nc.dram_tensor("fT", [F, NODES_C], bf, kind="ExternalInput")
    w0x_d = nc.dram_tensor("w0x", [P, P], bf, kind="ExternalInput")
    w0a_d = nc.dram_tensor("w0a", [P, P], bf, kind="ExternalInput")
    w0f_d = nc.dram_tensor("w0f", [F, P], bf, kind="ExternalInput")
    w1_d = nc.dram_tensor("w1", [P, P], bf, kind="ExternalInput")
    w2_d = nc.dram_tensor("w2", [P, P], bf, kind="ExternalInput")
    b0_d = nc.dram_tensor("b0", [P, 1], f32, kind="ExternalInput")
    b1_d = nc.dram_tensor("b1", [P, 1], f32, kind="ExternalInput")
    b2_d = nc.dram_tensor("b2", [P, 1], f32, kind="ExternalInput")
    out_d = nc.dram_tensor("out", [P, NODES_C], out_dt, kind="ExternalOutput")

    groups = [
        list(range(s, min(s + GROUP_SLOTS, SLOTS))) for s in range(0, SLOTS, GROUP_SLOTS)
    ]

    with tile.TileContext(nc) as tc:
        with (
            tc.tile_pool(name="const", bufs=1) as const_pool,
            tc.tile_pool(name="res", bufs=2) as res_pool,
            tc.tile_pool(name="attr", bufs=attr_bufs) as attr_pool,
            tc.tile_pool(name="oh", bufs=oh_bufs) as oh_pool,
            tc.tile_pool(name="aggp", bufs=4, space="PSUM") as aggp_pool,
            tc.tile_pool(name="mlpp", bufs=3, space="PSUM") as mlp_pool,
            tc.tile_pool(name="acts", bufs=2) as act_pool,
        ):
            # iota layout: "jt" = value j at col j*TPB+t (packed last AP dim
            # => DVE 2x mode, strided matmul rhs); "tj" = value j at col
            # t*W+j (contiguous matmul rhs, DVE 1x mode).
            iota_i = const_pool.tile([P, W * TPB], mybir.dt.int32)
            iota_pat = [[1, W], [0, TPB]] if oh_layout == "jt" else [[0, TPB], [1, W]]
            nc.gpsimd.iota(iota_i[:], pattern=iota_pat, base=0,
                           channel_multiplier=0)
            iota_b = const_pool.tile([P, W * TPB], bf)
            nc.vector.tensor_copy(iota_b[:], iota_i[:])

            w0x_t = const_pool.tile([P, P], bf)
            w0a_t = const_pool.tile([P, P], bf)
            w0f_t = const_pool.tile([F, P], bf)
            w1_t = const_pool.tile([P, P], bf)
            w2_t = const_pool.tile([P, P], bf)
            b0_t = const_pool.tile([P, 1], f32)
            b1_t = const_pool.tile([P, 1], f32)
            b2_t = const_pool.tile([P, 1], f32)
            for t, d_ in [(w0x_t, w0x_d), (w0a_t, w0a_d), (w0f_t, w0f_d),
                          (w1_t, w1_d), (w2_t, w2_d),
                          (b0_t, b0_d), (b1_t, b1_d), (b2_t, b2_d)]:
                nc.sync.dma_start(t[:], d_[:])

            def silu(hp, bias_t, g_w, tag):
                z = act_pool.tile([P, g_w], bf, tag=tag + "z")
                nc.scalar.activation(z[:], hp[:], mybir.ActivationFunctionType.Identity,
                                     bias=bias_t[:], scale=1.0)
                s = act_pool.tile([P, g_w], bf, tag=tag + "s")
                nc.scalar.activation(s[:], hp[:], mybir.ActivationFunctionType.Sigmoid,
                                     bias=bias_t[:], scale=1.0)
                h = act_pool.tile([P, g_w], bf, tag=tag + "h")
                nc.vector.tensor_tensor(out=h[:], in0=z[:], in1=s[:],
                                        op=mybir.AluOpType.mult)
                return h

            def body():
                dest_t = res_pool.tile([P, TT], bf, tag="dest")
                nc.sync.dma_start(dest_t[:], dest_d[:])
                xT_t = res_pool.tile([P, NODES_C], bf, tag="xT")
                nc.sync.dma_start(xT_t[:], xT_d[:])
                fT_t = res_pool.tile([F, NODES_C], bf, tag="fT")
                nc.sync.dma_start(fT_t[:], fT_d[:])

                chunk = {}
                ohbatch = {}

                def get_oh(k):
                    """one-hot [128 edges, W nodes] view for edge tile k."""
                    kb = k // TPB
                    if kb not in ohbatch:
                        nt = min(TPB, TT - kb * TPB)
                        t = oh_pool.tile([P, W * TPB], bf, tag="oh")
                        dst = dest_t[:, kb * TPB:kb * TPB + nt].to_broadcast([P, nt, W])
                        if oh_layout == "jt":
                            # reorder [p, t, j] -> [p, j, t]: packed t-dim last
                            dst = bass.AP(dst.tensor, dst.offset,
                                          [dst.ap[0], dst.ap[2], dst.ap[1]])
                            nc.vector.tensor_tensor(
                                out=t[:, :W * nt].rearrange("p (j t) -> p j t", t=nt),
                                in0=iota_b[:, :W * nt].rearrange("p (j t) -> p j t", t=nt),
                                in1=dst,
                                op=mybir.AluOpType.is_equal)
                        else:
                            nc.vector.tensor_tensor(
                                out=t[:, :W * nt].rearrange("p (t j) -> p t j", t=nt),
                                in0=iota_b[:, :W * nt].rearrange("p (t j) -> p t j", t=nt),
                                in1=dst,
                                op=mybir.AluOpType.is_equal)
                        ohbatch[kb] = t
                    if oh_layout == "jt":
                        tv = ohbatch[kb][:].rearrange("p (j t) -> p t j", t=TPB)
                        return tv[:, k % TPB, :]
                    return ohbatch[kb][:, (k % TPB) * W:(k % TPB + 1) * W]

                for gi, g in enumerate(groups):
                    g_w = len(g) * W
                    agg_sb = act_pool.tile([P, g_w], bf, tag="agg_sb")
                    for pi in range(0, len(g), SPP):
                        pair = g[pi:pi + SPP]
                        aggp = aggp_pool.tile([P, P], f32, space="PSUM")
                        for hi, s in enumerate(pair):
                            n_t = int(ts_real[s])
                            for j in range(n_t):
                                k = int(cum[s]) + j
                                ch = k // CH
                                if ch not in chunk:
                                    t = attr_pool.tile([P, CH * P], bf, tag="attr")
                                    nc.sync.dma_start(t[:], attr_d[ch])
                                    chunk[ch] = t
                                col = (k % CH) * P
                                nc.tensor.matmul(
                                    out=aggp[:, hi * W:(hi + 1) * W],
                                    lhsT=chunk[ch][:, col:col + P], rhs=get_oh(k),
                                    start=(j == 0), stop=(j == n_t - 1))
                        nc.scalar.copy(
                            agg_sb[:, pi * W:pi * W + len(pair) * W],
                            aggp[:, :len(pair) * W])

                    c0 = g[0] * W
                    h0p = mlp_pool.tile([P, g_w], f32, space="PSUM", tag="mlp")
                    nc.tensor.matmul(out=h0p[:], lhsT=w0x_t[:], rhs=xT_t[:, c0:c0 + g_w],
                                     start=True, stop=False)
                    nc.tensor.matmul(out=h0p[:], lhsT=w0a_t[:], rhs=agg_sb[:],
                                     start=False, stop=False)
                    nc.tensor.matmul(out=h0p[:], lhsT=w0f_t[:], rhs=fT_t[:, c0:c0 + g_w],
                                     start=False, stop=True)
                    h0 = silu(h0p, b0_t, g_w, "h0")
                    h1p = mlp_pool.tile([P, g_w], f32, space="PSUM", tag="mlp")
                    nc.tensor.matmul(out=h1p[:], lhsT=w1_t[:], rhs=h0[:], start=True, stop=True)
                    h1 = silu(h1p, b1_t, g_w, "h1")
                    outp = mlp_pool.tile([P, g_w], f32, space="PSUM", tag="mlp")
                    nc.tensor.matmul(out=outp[:], lhsT=w2_t[:], rhs=h1[:], start=True, stop=True)
                    ot = act_pool.tile([P, g_w], out_dt, tag="outt")
                    nc.scalar.activation(ot[:], outp[:], mybir.ActivationFunctionType.Identity,
                                         bias=b2_t[:], scale=1.0)
                    nc.sync.dma_start(out_d[:, c0:c0 + g_w], ot[:])

            if reps == 1:
                body()
            else:
                assert reps % bodies_per_iter == 0
                with tc.For_i(0, reps // bodies_per_iter, 1):
                    for _ in range(bodies_per_iter):
                        body()

    nc.compile()
    return nc


def _assemble(results, blocks_per_core):
    outT_full = np.zeros((P, NBLK, W), np.float32)
    for c in range(NCORES):
        outT_full[:, blocks_per_core[c], :] = np.asarray(
            results[c]["out"], np.float32).reshape(P, SLOTS, W)
    return np.ascontiguousarray(outT_full.reshape(P, NBLK * W)[:, :N].T)


def kernel(**inputs):
    from concourse.bass_utils import run_bass_kernel_spmd

    in_maps, blocks_per_core, TS, cum, NCH, ts_real = _prep(inputs)
    nc = _build(TS, cum, NCH, reps=int(os.environ.get("GNN_REPS", "1")), ts_real=ts_real)
    res = run_bass_kernel_spmd(nc, in_maps, core_ids=list(range(NCORES)))
    return _assemble(res.results, blocks_per_core)

